# revision 33
# baseline (speedup 1.0000x reference)
"""GRU-D decoder kernel for Trainium2 (8 NeuronCores, data-parallel over batch).

Math (mask == ones everywhere, which the reference hardcodes):
  x_hat = C (constant), d = dt broadcast, gamma_x unused.
  gamma[t,b,j] = exp(-relu(dt[t,b] * colsum(Wgh)[j] + bgh[j]))   (precomputed host-side)
  per step: hdec = gamma_t * h
            z = sigmoid(hdec @ Wz_h + Az0);  r = sigmoid(hdec @ Wr_h + Ar0)
            htl = tanh((r*hdec) @ Wh_h + Ah0)
            h = hdec + z*(htl - hdec)
  out[t] = h_t @ Wlin            (blin added host-side after the gather)
  where A?0 = C @ W?_x + colsum(W?_m) + b?  (time-constant, precomputed host-side).

Device layout: everything transposed (H on partitions as 4 tiles of 128,
batch=64 on the free dim), packed as SBUF tiles (128, 4*64) with column
index = kt*64 + b.  All state is bf16 (validated: global rel err ~5e-3).

v2 structure (vs the v1 baseline):
  - Per-step PE stream is r(16) z(16) htl(16, jo-major) proj(4, even steps)
    next-step psum inits(4).  The projection + inits fill the tanh/blend
    tail so the PE never idles long enough for the HAM clock gate to
    re-throttle (v1 oscillated 1.2<->2.4 GHz the whole run).
  - Projection batches TWO timesteps per weight pass: lhsT = h ring slots
    (t, t+1) giving M=128, rhs = Wlin tiles at N=512.  5 MMs/step -> 2.
  - All gate activations output bf16; the h state is a bf16 ring buffer
    (4 slots) read directly as the projection's stationary operand, so the
    v1 per-step fp32 state + hbf copy + separate osb copy disappear.
  - ph0/ph1 psum pools are double-buffered so next-step inits never wait
    on the current tanh reads.
"""

import numpy as np
import ml_dtypes

T, B, H, O = 100, 512, 512, 512
NCORES = 8
BL = B // NCORES  # 64
KC = 4  # contraction chunks of 128
JT = 4  # output j-tiles of 128
FR = JT * BL  # 256
HB = FR // 2  # 128 (half of the free dim; = 2 j-tiles)
GCH = 20  # gamma chunk (steps per DMA)
PSB = 512  # psum bank width in fp32

_BUILD_CACHE = {}


def _build_program():
    if "nc" in _BUILD_CACHE:
        return _BUILD_CACHE["nc"]

    import concourse.tile as tile
    import concourse.mybir as mybir
    from concourse import bacc
    from contextlib import ExitStack

    f32 = mybir.dt.float32
    bf16 = mybir.dt.bfloat16
    AF = mybir.ActivationFunctionType

    nc = bacc.Bacc("TRN2", target_bir_lowering=False, debug=False,
                   num_devices=NCORES)

    gam_d = nc.dram_tensor("gam", [128, T, FR], bf16, kind="ExternalInput")
    wzr_d = nc.dram_tensor("wzr", [128, KC * 2 * JT * 128], bf16, kind="ExternalInput")
    wht_d = nc.dram_tensor("wht", [128, KC * JT * 128], bf16, kind="ExternalInput")
    wlin_d = nc.dram_tensor("wlin", [128, KC * O], bf16, kind="ExternalInput")
    a0z_d = nc.dram_tensor("a0z", [128, FR], bf16, kind="ExternalInput")
    a0r_d = nc.dram_tensor("a0r", [128, FR], bf16, kind="ExternalInput")
    a0h_d = nc.dram_tensor("a0h", [128, FR], bf16, kind="ExternalInput")
    ident_d = nc.dram_tensor("ident", [128, 128], bf16, kind="ExternalInput")
    out_d = nc.dram_tensor("out", [T, BL, O], f32, kind="ExternalOutput")

    with tile.TileContext(nc) as tc, ExitStack() as ctx:
        constp = ctx.enter_context(tc.tile_pool(name="const", bufs=1))
        gpool = ctx.enter_context(tc.tile_pool(name="gam", bufs=2))
        stgp = ctx.enter_context(tc.tile_pool(name="stg", bufs=3))
        hdp = ctx.enter_context(tc.tile_pool(name="hd", bufs=2))
        actp = ctx.enter_context(tc.tile_pool(name="act", bufs=2))
        osbp = ctx.enter_context(tc.tile_pool(name="osb", bufs=2))
        prp0 = ctx.enter_context(tc.tile_pool(name="pr0", bufs=1, space="PSUM"))
        prp1 = ctx.enter_context(tc.tile_pool(name="pr1", bufs=1, space="PSUM"))
        pzp = ctx.enter_context(tc.tile_pool(name="pz", bufs=1, space="PSUM"))
        php0 = ctx.enter_context(tc.tile_pool(name="ph0", bufs=1, space="PSUM"))
        php1 = ctx.enter_context(tc.tile_pool(name="ph1", bufs=1, space="PSUM"))
        pjp = ctx.enter_context(tc.tile_pool(name="pj", bufs=2, space="PSUM"))
        scrp = ctx.enter_context(tc.tile_pool(name="scr", bufs=1, space="PSUM"))

        wzr = constp.tile([128, KC * 2 * JT * 128], bf16)
        half = KC * JT * 128
        nc.sync.dma_start(wzr[:, 0:half], wzr_d[:, 0:half])
        nc.sync.dma_start(wzr[:, half:2 * half], wzr_d[:, half:2 * half])
        wht = constp.tile([128, KC * JT * 128], bf16)
        nc.sync.dma_start(wht[:], wht_d[:])
        wlin = constp.tile([128, KC * O], bf16)
        nc.sync.dma_start(wlin[:], wlin_d[:])
        a0z = constp.tile([128, FR], bf16)
        nc.sync.dma_start(a0z[:], a0z_d[:])
        a0r = constp.tile([128, FR], bf16)
        nc.sync.dma_start(a0r[:], a0r_d[:])
        a0h = constp.tile([128, FR], bf16)
        nc.sync.dma_start(a0h[:], a0h_d[:])
        ident = constp.tile([128, 128], bf16)
        nc.sync.dma_start(ident[:], ident_d[:])

        # Projection staging: a per-pair tile [128, (kt, parity*BL+b)] written
        # by one off-critical-path copy per step, so the matmul reads are
        # contiguous 2D slices AND the blend's h writes never alias the
        # projection reads (address-overlap tracking would serialize them).

        def wzr_blk(g, jo, kc):
            i = ((kc * 2 + g) * JT + jo) * 128
            return wzr[:, i:i + 128]

        def wht_blk(jo, kc):
            i = (kc * JT + jo) * 128
            return wht[:, i:i + 128]

        # gamma chunks, preloaded half a chunk ahead
        chunks = {}

        def ensure_chunk(c):
            if c in chunks or c * GCH >= T:
                return
            t0 = c * GCH
            t1 = min(t0 + GCH, T)
            gt = gpool.tile([128, GCH * FR], bf16, tag="gchunk")
            nc.sync.dma_start(gt[:, 0:(t1 - t0) * FR], gam_d[:, t0:t1, :])
            chunks[c] = gt

        def gamma_half(tt, hf):
            c2, o2 = divmod(tt, GCH)
            return chunks[c2][:, o2 * FR + hf * HB: o2 * FR + (hf + 1) * HB]

        ensure_chunk(0)

        # step-0 decayed state is zero
        hd = hdp.tile([128, FR], bf16, tag="hd")
        nc.vector.memset(hd[:], 0.0)

        # scratch psum bank for warm-keeper matmuls (results never read; they
        # only keep the PE's HAM clock-gate at 2.4 GHz through the per-step
        # activation/blend windows)
        scr = scrp.tile([128, PSB], f32)

        def dummy_mm():
            nc.tensor.matmul(scr[:, 0:FR], ident[:], a0z[:], start=True, stop=True)

        def make_inits():
            """Allocate next step's psum tiles and preload the gate constants
            (identity matmuls run at the end of the previous PE stream).
            The r preactivation is split across two banks so sigmoid(r) can
            start at the r-block midpoint."""
            pr0 = prp0.tile([128, PSB], f32, tag="pr0")
            nc.tensor.matmul(pr0[:, 0:HB], ident[:], a0r[:, 0:HB], start=True, stop=False)
            pr1 = prp1.tile([128, PSB], f32, tag="pr1")
            nc.tensor.matmul(pr1[:, 0:HB], ident[:], a0r[:, HB:FR], start=True, stop=False)
            pz = pzp.tile([128, PSB], f32, tag="pz")
            nc.tensor.matmul(pz[:, 0:FR], ident[:], a0z[:], start=True, stop=False)
            ph0 = php0.tile([128, PSB], f32, tag="ph0")
            nc.tensor.matmul(ph0[:, 0:HB], ident[:], a0h[:, 0:HB], start=True, stop=False)
            ph1 = php1.tile([128, PSB], f32, tag="ph1")
            nc.tensor.matmul(ph1[:, 0:HB], ident[:], a0h[:, HB:FR], start=True, stop=False)
            return pr0, pr1, pz, ph0, ph1

        def issue_proj(stg, pj, kcs):
            """Project a staged h pair: accumulating matmuls with M=128 (two
            steps x 64 batch), N=512.  Split across two scan steps
            (kcs=(0,1) then (2,3)) so both steps' PE tails get fill work."""
            for kc in kcs:
                nc.tensor.matmul(
                    pj[:],
                    stg[:, kc, :],
                    wlin[:, kc * O:(kc + 1) * O],
                    start=(kc == 0), stop=(kc == KC - 1),
                )

        def evac_proj(t0, pj):
            # negative offset = LOWER priority: the scheduler must never slot
            # these copies ahead of the blend chain that gates the next step
            with tc.high_priority(offset=-400):
                osb = osbp.tile([128, O], f32, tag="osb")
                nc.scalar.copy(osb[:, 0:256], pj[:, 0:256])
                nc.scalar.copy(osb[:, 256:512], pj[:, 256:512])
                nc.sync.dma_start(out_d[t0:t0 + 2], osb[:])

        pr0, pr1, pz, ph0, ph1 = make_inits()
        pj_cur = None
        evac_pending = None
        stg_cur = stg_prev = None

        for t in range(T):
            c, o = divmod(t, GCH)
            if o == GCH // 2:
                ensure_chunk(c + 1)

            # ---- r gate matmuls: jo-half-major (pr0 stops after 8 MMs so
            # sigmoid(r) half 0 starts at the r-block midpoint), kc-outer
            # within each half so they start on partial hd
            for prx, job in ((pr0, 0), (pr1, 2)):
                for kc in range(KC):
                    for jo in (job, job + 1):
                        nc.tensor.matmul(
                            prx[:, (jo - job) * BL:(jo - job + 1) * BL],
                            wzr_blk(1, jo, kc),
                            hd[:, kc * BL:(kc + 1) * BL],
                            start=False, stop=(kc == KC - 1),
                        )
            # ---- z gate matmuls (fill the sigmoid(r)/rh window)
            for kc in range(KC):
                for jo in range(JT):
                    nc.tensor.matmul(
                        pz[:, jo * BL:(jo + 1) * BL],
                        wzr_blk(0, jo, kc),
                        hd[:, kc * BL:(kc + 1) * BL],
                        start=False, stop=(kc == KC - 1),
                    )
            # ---- sigmoid(r) and r*hd in halves
            rb = actp.tile([128, FR], bf16, tag="rb")
            nc.scalar.activation(rb[:, 0:HB], pr0[:, 0:HB], AF.Sigmoid)
            nc.scalar.activation(rb[:, HB:FR], pr1[:, 0:HB], AF.Sigmoid)
            rh = hdp.tile([128, FR], bf16, tag="rh")
            nc.vector.tensor_mul(rh[:, 0:HB], rb[:, 0:HB], hd[:, 0:HB])
            nc.vector.tensor_mul(rh[:, HB:FR], rb[:, HB:FR], hd[:, HB:FR])

            # ---- drain last step's finished projection pair here: the ACT
            # copies land in the z/htl-matmul window instead of queueing ahead
            # of the next step's activations
            if evac_pending is not None:
                evac_proj(*evac_pending)
                evac_pending = None

            # ---- candidate gate in three phases: kc 0,1 for all jo (needs
            # only rh half 0), then kc 2,3 for jo 0,1 (ph0 stops 4 MMs before
            # the block end, so tanh half 0 starts early), then kc 2,3 for
            # jo 2,3 (ph1 stops)
            def htl_mm(jo, kc):
                tgt, col = (ph0, jo) if jo < 2 else (ph1, jo - 2)
                nc.tensor.matmul(
                    tgt[:, col * BL:(col + 1) * BL],
                    wht_blk(jo, kc),
                    rh[:, kc * BL:(kc + 1) * BL],
                    start=False, stop=(kc == KC - 1),
                )
            for kc in (0, 1):
                for jo in range(JT):
                    htl_mm(jo, kc)
            # warm-keeper: fills the PE if rh half 1 is still in flight
            dummy_mm()
            for jo in (0, 1):
                for kc in (2, 3):
                    htl_mm(jo, kc)
            for jo in (2, 3):
                for kc in (2, 3):
                    htl_mm(jo, kc)
            zf = actp.tile([128, FR], bf16, tag="zf")
            nc.scalar.activation(zf[:], pz[:, 0:FR], AF.Sigmoid)
            # (1-z)*hd, computed in the htl window so the post-tanh chain is
            # only mul-add-mul
            bb = actp.tile([128, FR], bf16, tag="bb")
            nc.vector.tensor_mul(bb[:], zf[:], hd[:])
            bq = actp.tile([128, FR], bf16, tag="bq")
            nc.vector.tensor_sub(bq[:], hd[:], bb[:])

            # ---- tail fill on PE: half a pair-projection every step, plus
            # warm-keeper matmuls so the HAM clock gate never sees an idle
            # window during the tanh/blend tail
            dummy_mm()
            if t >= 2 and t % 2 == 0:
                stg_prev = stg_cur
                pj_cur = pjp.tile([128, PSB], f32, tag="pj")
                issue_proj(stg_prev, pj_cur, (0, 1))
            elif t >= 3 and t % 2 == 1:
                issue_proj(stg_prev, pj_cur, (2, 3))
            ph0_r, ph1_r = ph0, ph1
            if t + 1 < T:
                pr0, pr1, pz, ph0, ph1 = make_inits()
                dummy_mm()

            # ---- tanh + blend (h = (1-z)*hd + z*htl), then decay for t+1.
            # Post-tanh chain per half: hv = z*htl; hh = bq + hv; hdn = g*hh.
            hd_n = None
            if t + 1 < T:
                hd_n = hdp.tile([128, FR], bf16, tag="hd")
            hh = hdp.tile([128, FR], bf16, tag="hh")
            for hf, ph in ((0, ph0_r), (1, ph1_r)):
                sl = slice(hf * HB, (hf + 1) * HB)
                htl = actp.tile([128, HB], bf16, tag=f"htl{hf}")
                nc.scalar.activation(htl[:], ph[:, 0:HB], AF.Tanh)
                hv = actp.tile([128, HB], bf16, tag=f"hv{hf}")
                nc.vector.tensor_mul(hv[:], zf[:, sl], htl[:])
                nc.vector.tensor_add(hh[:, sl], bq[:, sl], hv[:])
                if t + 1 < T:
                    nc.vector.tensor_mul(
                        hd_n[:, sl],
                        chunks[(t + 1) // GCH][
                            :, ((t + 1) % GCH) * FR + hf * HB:
                               ((t + 1) % GCH) * FR + (hf + 1) * HB],
                        hh[:, sl])
            if t + 1 < T:
                hd = hd_n

            # ---- stage h(t) for the pair projection: mildly de-prioritized
            # so it lands in the next step's gate window, clear of both the
            # blend chain and (3 steps later) its own buffer's proj readers
            if t % 2 == 0:
                stg_cur = stgp.tile([128, KC, 2 * BL], bf16, tag="stg")
            with tc.high_priority(offset=-60):
                nc.vector.tensor_copy(
                    stg_cur[:, :, (t % 2) * BL:(t % 2 + 1) * BL], hh[:])

            # ---- mark the finished projection pair for draining next step
            if t >= 3 and t % 2 == 1:
                evac_pending = (t - 3, pj_cur)

        if evac_pending is not None:
            evac_proj(*evac_pending)
        # final pair (T-2, T-1)
        pj_cur = pjp.tile([128, PSB], f32, tag="pj")
        issue_proj(stg_cur, pj_cur, (0, 1, 2, 3))
        evac_proj(T - 2, pj_cur)

    nc.compile()
    _BUILD_CACHE["nc"] = nc
    return nc


def _host_prep(C, t, Wz, bz, Wr, br, Wh, bh, Wgh, bgh, Wlin):
    """Build per-core input maps (all the precomputed, packed device tensors)."""
    bf = ml_dtypes.bfloat16

    s = Wgh.sum(axis=0)  # (H,)
    t3 = t[:, :, 0]  # (T,B)
    dt = np.concatenate([np.zeros((1, B), np.float32), t3[1:] - t3[:-1]], axis=0)
    # gamma (T,B,H)
    gam = np.exp(-np.maximum(dt[:, :, None] * s[None, None, :] + bgh[None, None, :], 0.0)).astype(np.float32)

    def gate_const(W, b):
        # C @ W_x + colsum(W_m) + b  -> (B,H)
        return C @ W[0:H] + (W[2 * H:3 * H].sum(axis=0) + b)[None, :]

    Az0 = gate_const(Wz, bz).astype(np.float32)
    Ar0 = gate_const(Wr, br).astype(np.float32)
    Ah0 = gate_const(Wh, bh).astype(np.float32)

    Wg = np.stack([Wz[H:2 * H], Wr[H:2 * H]])  # (2,H,H)
    # wzr packed: [k, (kc,g,jo,m)]
    wzr = Wg.reshape(2, KC, 128, JT, 128).transpose(2, 1, 0, 3, 4).reshape(128, KC * 2 * JT * 128)
    wht = Wh[H:2 * H].reshape(KC, 128, JT, 128).transpose(1, 0, 2, 3).reshape(128, KC * JT * 128)
    wlin = Wlin.reshape(KC, 128, O).transpose(1, 0, 2).reshape(128, KC * O)
    wzr = np.ascontiguousarray(wzr, dtype=bf)
    wht = np.ascontiguousarray(wht, dtype=bf)
    wlin = np.ascontiguousarray(wlin, dtype=bf)
    ident = np.eye(128, dtype=bf)

    in_maps = []
    for i in range(NCORES):
        sl = slice(i * BL, (i + 1) * BL)
        gf = gam[:, sl, :]  # (T,BL,H)
        # gam packed: [p, t, kt*BL+b]
        gp = np.ascontiguousarray(
            gf.reshape(T, BL, KC, 128).transpose(3, 0, 2, 1).reshape(128, T, KC * BL),
            dtype=bf)

        def packA(A):
            return np.ascontiguousarray(
                A[sl].reshape(BL, JT, 128).transpose(2, 1, 0).reshape(128, JT * BL), dtype=bf)

        in_maps.append({
            "gam": gp,
            "wzr": wzr,
            "wht": wht,
            "wlin": wlin,
            "a0z": packA(Az0),
            "a0r": packA(Ar0),
            "a0h": packA(Ah0),
            "ident": ident,
        })
    return in_maps


def kernel(C, t, mask, Wz, bz, Wr, br, Wh, bh, Wgh, bgh, wgx, bgx, Wlin, blin,
           _trace=False, _trace_kwargs=None):
    C = np.asarray(C, np.float32)
    t = np.asarray(t, np.float32)
    nc = _build_program()
    in_maps = _host_prep(C, t,
                         np.asarray(Wz, np.float32), np.asarray(bz, np.float32),
                         np.asarray(Wr, np.float32), np.asarray(br, np.float32),
                         np.asarray(Wh, np.float32), np.asarray(bh, np.float32),
                         np.asarray(Wgh, np.float32), np.asarray(bgh, np.float32),
                         np.asarray(Wlin, np.float32))

    from concourse.bass_utils import run_bass_kernel_spmd
    res = run_bass_kernel_spmd(nc, in_maps, list(range(NCORES)),
                               trace=_trace, **(_trace_kwargs or {}))
    outs = [res.results[i]["out"] for i in range(NCORES)]
    full = np.concatenate(outs, axis=1).astype(np.float32)  # (T,B,O)
    full += np.asarray(blin, np.float32)[None, None, :]
    kernel._last_results = res
    return full


# revision 34
# speedup vs baseline: 1.0262x; 1.0262x over previous
"""GRU-D decoder kernel for Trainium2 (8 NeuronCores, data-parallel over batch).

Math (mask == ones everywhere, which the reference hardcodes):
  x_hat = C (constant), d = dt broadcast, gamma_x unused.
  gamma[t,b,j] = exp(-relu(dt[t,b] * colsum(Wgh)[j] + bgh[j]))   (precomputed host-side)
  per step: hdec = gamma_t * h
            z = sigmoid(hdec @ Wz_h + Az0);  r = sigmoid(hdec @ Wr_h + Ar0)
            htl = tanh((r*hdec) @ Wh_h + Ah0)
            h = hdec + z*(htl - hdec)
  out[t] = h_t @ Wlin            (blin added host-side after the gather)
  where A?0 = C @ W?_x + colsum(W?_m) + b?  (time-constant, precomputed host-side).

Device layout: everything transposed (H on partitions as 4 tiles of 128,
batch=64 on the free dim), packed as SBUF tiles (128, 4*64) with column
index = kt*64 + b.  All state is bf16 (validated: global rel err ~5e-3).

v2 structure (vs the v1 baseline):
  - Per-step PE stream is r(16) z(16) htl(16, jo-major) proj(4, even steps)
    next-step psum inits(4).  The projection + inits fill the tanh/blend
    tail so the PE never idles long enough for the HAM clock gate to
    re-throttle (v1 oscillated 1.2<->2.4 GHz the whole run).
  - Projection batches TWO timesteps per weight pass: lhsT = h ring slots
    (t, t+1) giving M=128, rhs = Wlin tiles at N=512.  5 MMs/step -> 2.
  - All gate activations output bf16; the h state is a bf16 ring buffer
    (4 slots) read directly as the projection's stationary operand, so the
    v1 per-step fp32 state + hbf copy + separate osb copy disappear.
  - ph0/ph1 psum pools are double-buffered so next-step inits never wait
    on the current tanh reads.
"""

import numpy as np
import ml_dtypes

T, B, H, O = 100, 512, 512, 512
NCORES = 8
BL = B // NCORES  # 64
KC = 4  # contraction chunks of 128
JT = 4  # output j-tiles of 128
FR = JT * BL  # 256
HB = FR // 2  # 128 (half of the free dim; = 2 j-tiles)
GCH = 20  # gamma chunk (steps per DMA)
PSB = 512  # psum bank width in fp32

_BUILD_CACHE = {}


def _build_program():
    if "nc" in _BUILD_CACHE:
        return _BUILD_CACHE["nc"]

    import concourse.tile as tile
    import concourse.mybir as mybir
    from concourse import bacc
    from contextlib import ExitStack

    f32 = mybir.dt.float32
    bf16 = mybir.dt.bfloat16
    AF = mybir.ActivationFunctionType

    nc = bacc.Bacc("TRN2", target_bir_lowering=False, debug=False,
                   num_devices=NCORES)

    gam_d = nc.dram_tensor("gam", [128, T, FR], bf16, kind="ExternalInput")
    wzr_d = nc.dram_tensor("wzr", [128, KC * 2 * JT * 128], bf16, kind="ExternalInput")
    wht_d = nc.dram_tensor("wht", [128, KC * JT * 128], bf16, kind="ExternalInput")
    wlin_d = nc.dram_tensor("wlin", [128, KC * O], bf16, kind="ExternalInput")
    a0z_d = nc.dram_tensor("a0z", [128, FR], bf16, kind="ExternalInput")
    a0r_d = nc.dram_tensor("a0r", [128, FR], bf16, kind="ExternalInput")
    a0h_d = nc.dram_tensor("a0h", [128, FR], bf16, kind="ExternalInput")
    ident_d = nc.dram_tensor("ident", [128, 128], bf16, kind="ExternalInput")
    out_d = nc.dram_tensor("out", [T, BL, O], f32, kind="ExternalOutput")

    with tile.TileContext(nc) as tc, ExitStack() as ctx:
        constp = ctx.enter_context(tc.tile_pool(name="const", bufs=1))
        gpool = ctx.enter_context(tc.tile_pool(name="gam", bufs=2))
        stgp = ctx.enter_context(tc.tile_pool(name="stg", bufs=3))
        hdp = ctx.enter_context(tc.tile_pool(name="hd", bufs=2))
        actp = ctx.enter_context(tc.tile_pool(name="act", bufs=2))
        osbp = ctx.enter_context(tc.tile_pool(name="osb", bufs=2))
        prp0 = ctx.enter_context(tc.tile_pool(name="pr0", bufs=1, space="PSUM"))
        prp1 = ctx.enter_context(tc.tile_pool(name="pr1", bufs=1, space="PSUM"))
        pzp = ctx.enter_context(tc.tile_pool(name="pz", bufs=1, space="PSUM"))
        php0 = ctx.enter_context(tc.tile_pool(name="ph0", bufs=1, space="PSUM"))
        php1 = ctx.enter_context(tc.tile_pool(name="ph1", bufs=1, space="PSUM"))
        pjp = ctx.enter_context(tc.tile_pool(name="pj", bufs=2, space="PSUM"))
        scrp = ctx.enter_context(tc.tile_pool(name="scr", bufs=1, space="PSUM"))

        wzr = constp.tile([128, KC * 2 * JT * 128], bf16)
        nc.sync.dma_start(wzr[:], wzr_d[:])
        wht = constp.tile([128, KC * JT * 128], bf16)
        nc.sync.dma_start(wht[:], wht_d[:])
        wlin = constp.tile([128, KC * O], bf16)
        nc.sync.dma_start(wlin[:], wlin_d[:])
        a0z = constp.tile([128, FR], bf16)
        nc.sync.dma_start(a0z[:], a0z_d[:])
        a0r = constp.tile([128, FR], bf16)
        nc.sync.dma_start(a0r[:], a0r_d[:])
        a0h = constp.tile([128, FR], bf16)
        nc.sync.dma_start(a0h[:], a0h_d[:])
        ident = constp.tile([128, 128], bf16)
        nc.sync.dma_start(ident[:], ident_d[:])

        # Projection staging: a per-pair tile [128, (kt, parity*BL+b)] written
        # by one off-critical-path copy per step, so the matmul reads are
        # contiguous 2D slices AND the blend's h writes never alias the
        # projection reads (address-overlap tracking would serialize them).

        def wzr_blk(g, jo, kc):
            i = ((kc * 2 + g) * JT + jo) * 128
            return wzr[:, i:i + 128]

        def wht_blk(jo, kc):
            i = (kc * JT + jo) * 128
            return wht[:, i:i + 128]

        # gamma chunks, preloaded half a chunk ahead
        chunks = {}

        def ensure_chunk(c):
            if c in chunks or c * GCH >= T:
                return
            t0 = c * GCH
            t1 = min(t0 + GCH, T)
            gt = gpool.tile([128, GCH * FR], bf16, tag="gchunk")
            nc.sync.dma_start(gt[:, 0:(t1 - t0) * FR], gam_d[:, t0:t1, :])
            chunks[c] = gt

        def gamma_half(tt, hf):
            c2, o2 = divmod(tt, GCH)
            return chunks[c2][:, o2 * FR + hf * HB: o2 * FR + (hf + 1) * HB]

        ensure_chunk(0)

        # step-0 decayed state is zero
        hd = hdp.tile([128, FR], bf16, tag="hd")
        nc.vector.memset(hd[:], 0.0)

        # scratch psum bank for warm-keeper matmuls (results never read; they
        # only keep the PE's HAM clock-gate at 2.4 GHz through the per-step
        # activation/blend windows)
        scr = scrp.tile([128, PSB], f32)

        def dummy_mm():
            nc.tensor.matmul(scr[:, 0:FR], ident[:], a0z[:], start=True, stop=True)

        def make_inits():
            """Allocate next step's psum tiles and preload the gate constants
            (identity matmuls run at the end of the previous PE stream).
            The r preactivation is split across two banks so sigmoid(r) can
            start at the r-block midpoint."""
            pr0 = prp0.tile([128, PSB], f32, tag="pr0")
            nc.tensor.matmul(pr0[:, 0:HB], ident[:], a0r[:, 0:HB], start=True, stop=False)
            pr1 = prp1.tile([128, PSB], f32, tag="pr1")
            nc.tensor.matmul(pr1[:, 0:HB], ident[:], a0r[:, HB:FR], start=True, stop=False)
            pz = pzp.tile([128, PSB], f32, tag="pz")
            nc.tensor.matmul(pz[:, 0:FR], ident[:], a0z[:], start=True, stop=False)
            ph0 = php0.tile([128, PSB], f32, tag="ph0")
            nc.tensor.matmul(ph0[:, 0:HB], ident[:], a0h[:, 0:HB], start=True, stop=False)
            ph1 = php1.tile([128, PSB], f32, tag="ph1")
            nc.tensor.matmul(ph1[:, 0:HB], ident[:], a0h[:, HB:FR], start=True, stop=False)
            return pr0, pr1, pz, ph0, ph1

        def issue_proj(stg, pj, kcs):
            """Project a staged h pair: accumulating matmuls with M=128 (two
            steps x 64 batch), N=512.  Split across two scan steps
            (kcs=(0,1) then (2,3)) so both steps' PE tails get fill work."""
            for kc in kcs:
                nc.tensor.matmul(
                    pj[:],
                    stg[:, kc, :],
                    wlin[:, kc * O:(kc + 1) * O],
                    start=(kc == 0), stop=(kc == KC - 1),
                )

        def evac_proj(t0, pj):
            # negative offset = LOWER priority: the scheduler must never slot
            # these copies ahead of the blend chain that gates the next step
            with tc.high_priority(offset=-400):
                osb = osbp.tile([128, O], f32, tag="osb")
                nc.scalar.copy(osb[:, 0:256], pj[:, 0:256])
                nc.scalar.copy(osb[:, 256:512], pj[:, 256:512])
                nc.sync.dma_start(out_d[t0:t0 + 2], osb[:])

        pr0, pr1, pz, ph0, ph1 = make_inits()
        pj_cur = None
        evac_pending = None
        stg_cur = stg_prev = None

        for t in range(T):
            c, o = divmod(t, GCH)
            if o == GCH // 2:
                ensure_chunk(c + 1)

            # ---- r gate matmuls: jo-half-major (pr0 stops after 8 MMs so
            # sigmoid(r) half 0 starts at the r-block midpoint), kc-outer
            # within each half so they start on partial hd
            for prx, job in ((pr0, 0), (pr1, 2)):
                for kc in range(KC):
                    for jo in (job, job + 1):
                        nc.tensor.matmul(
                            prx[:, (jo - job) * BL:(jo - job + 1) * BL],
                            wzr_blk(1, jo, kc),
                            hd[:, kc * BL:(kc + 1) * BL],
                            start=False, stop=(kc == KC - 1),
                        )
            # ---- z gate matmuls (fill the sigmoid(r)/rh window)
            for kc in range(KC):
                for jo in range(JT):
                    nc.tensor.matmul(
                        pz[:, jo * BL:(jo + 1) * BL],
                        wzr_blk(0, jo, kc),
                        hd[:, kc * BL:(kc + 1) * BL],
                        start=False, stop=(kc == KC - 1),
                    )
            # ---- sigmoid(r) and r*hd in halves
            rb = actp.tile([128, FR], bf16, tag="rb")
            nc.scalar.activation(rb[:, 0:HB], pr0[:, 0:HB], AF.Sigmoid)
            nc.scalar.activation(rb[:, HB:FR], pr1[:, 0:HB], AF.Sigmoid)
            rh = hdp.tile([128, FR], bf16, tag="rh")
            nc.vector.tensor_mul(rh[:, 0:HB], rb[:, 0:HB], hd[:, 0:HB])
            nc.vector.tensor_mul(rh[:, HB:FR], rb[:, HB:FR], hd[:, HB:FR])

            # ---- drain last step's finished projection pair here: the ACT
            # copies land in the z/htl-matmul window instead of queueing ahead
            # of the next step's activations
            if evac_pending is not None:
                evac_proj(*evac_pending)
                evac_pending = None

            # ---- candidate gate in three phases: kc 0,1 for all jo (needs
            # only rh half 0), then kc 2,3 for jo 0,1 (ph0 stops 4 MMs before
            # the block end, so tanh half 0 starts early), then kc 2,3 for
            # jo 2,3 (ph1 stops)
            def htl_mm(jo, kc):
                tgt, col = (ph0, jo) if jo < 2 else (ph1, jo - 2)
                nc.tensor.matmul(
                    tgt[:, col * BL:(col + 1) * BL],
                    wht_blk(jo, kc),
                    rh[:, kc * BL:(kc + 1) * BL],
                    start=False, stop=(kc == KC - 1),
                )
            for kc in (0, 1):
                for jo in range(JT):
                    htl_mm(jo, kc)
            for jo in (0, 1):
                for kc in (2, 3):
                    htl_mm(jo, kc)
            for jo in (2, 3):
                for kc in (2, 3):
                    htl_mm(jo, kc)
            zf = actp.tile([128, FR], bf16, tag="zf")
            nc.scalar.activation(zf[:], pz[:, 0:FR], AF.Sigmoid)
            # (1-z)*hd, computed in the htl window so the post-tanh chain is
            # only mul-add-mul
            bb = actp.tile([128, FR], bf16, tag="bb")
            nc.vector.tensor_mul(bb[:], zf[:], hd[:])
            bq = actp.tile([128, FR], bf16, tag="bq")
            nc.vector.tensor_sub(bq[:], hd[:], bb[:])

            # ---- tail fill on PE: half a pair-projection every step, plus
            # warm-keeper matmuls so the HAM clock gate never sees an idle
            # window during the tanh/blend tail
            dummy_mm()
            if t >= 2 and t % 2 == 0:
                stg_prev = stg_cur
                pj_cur = pjp.tile([128, PSB], f32, tag="pj")
                issue_proj(stg_prev, pj_cur, (0, 1))
            elif t >= 3 and t % 2 == 1:
                issue_proj(stg_prev, pj_cur, (2, 3))
            ph0_r, ph1_r = ph0, ph1
            if t + 1 < T:
                pr0, pr1, pz, ph0, ph1 = make_inits()
                dummy_mm()

            # ---- tanh + blend (h = (1-z)*hd + z*htl), then decay for t+1.
            # Post-tanh chain per half: hv = z*htl; hh = bq + hv; hdn = g*hh.
            hd_n = None
            if t + 1 < T:
                hd_n = hdp.tile([128, FR], bf16, tag="hd")
            hh = hdp.tile([128, FR], bf16, tag="hh")
            for hf, ph in ((0, ph0_r), (1, ph1_r)):
                sl = slice(hf * HB, (hf + 1) * HB)
                htl = actp.tile([128, HB], bf16, tag=f"htl{hf}")
                nc.scalar.activation(htl[:], ph[:, 0:HB], AF.Tanh)
                hv = actp.tile([128, HB], bf16, tag=f"hv{hf}")
                nc.vector.tensor_mul(hv[:], zf[:, sl], htl[:])
                nc.vector.tensor_add(hh[:, sl], bq[:, sl], hv[:])
                if t + 1 < T:
                    nc.vector.tensor_mul(
                        hd_n[:, sl],
                        chunks[(t + 1) // GCH][
                            :, ((t + 1) % GCH) * FR + hf * HB:
                               ((t + 1) % GCH) * FR + (hf + 1) * HB],
                        hh[:, sl])
            if t + 1 < T:
                hd = hd_n

            # ---- stage h(t) for the pair projection: mildly de-prioritized
            # so it lands in the next step's gate window, clear of both the
            # blend chain and (3 steps later) its own buffer's proj readers
            if t % 2 == 0:
                stg_cur = stgp.tile([128, KC, 2 * BL], bf16, tag="stg")
            with tc.high_priority(offset=-60):
                nc.vector.tensor_copy(
                    stg_cur[:, :, (t % 2) * BL:(t % 2 + 1) * BL], hh[:])

            # ---- mark the finished projection pair for draining next step
            if t >= 3 and t % 2 == 1:
                evac_pending = (t - 3, pj_cur)

        if evac_pending is not None:
            evac_proj(*evac_pending)
        # final pair (T-2, T-1)
        pj_cur = pjp.tile([128, PSB], f32, tag="pj")
        issue_proj(stg_cur, pj_cur, (0, 1, 2, 3))
        evac_proj(T - 2, pj_cur)

    nc.compile()
    _BUILD_CACHE["nc"] = nc
    return nc


def _host_prep(C, t, Wz, bz, Wr, br, Wh, bh, Wgh, bgh, Wlin):
    """Build per-core input maps (all the precomputed, packed device tensors)."""
    bf = ml_dtypes.bfloat16

    s = Wgh.sum(axis=0)  # (H,)
    t3 = t[:, :, 0]  # (T,B)
    dt = np.concatenate([np.zeros((1, B), np.float32), t3[1:] - t3[:-1]], axis=0)
    # gamma (T,B,H)
    gam = np.exp(-np.maximum(dt[:, :, None] * s[None, None, :] + bgh[None, None, :], 0.0)).astype(np.float32)

    def gate_const(W, b):
        # C @ W_x + colsum(W_m) + b  -> (B,H)
        return C @ W[0:H] + (W[2 * H:3 * H].sum(axis=0) + b)[None, :]

    Az0 = gate_const(Wz, bz).astype(np.float32)
    Ar0 = gate_const(Wr, br).astype(np.float32)
    Ah0 = gate_const(Wh, bh).astype(np.float32)

    Wg = np.stack([Wz[H:2 * H], Wr[H:2 * H]])  # (2,H,H)
    # wzr packed: [k, (kc,g,jo,m)]
    wzr = Wg.reshape(2, KC, 128, JT, 128).transpose(2, 1, 0, 3, 4).reshape(128, KC * 2 * JT * 128)
    wht = Wh[H:2 * H].reshape(KC, 128, JT, 128).transpose(1, 0, 2, 3).reshape(128, KC * JT * 128)
    wlin = Wlin.reshape(KC, 128, O).transpose(1, 0, 2).reshape(128, KC * O)
    wzr = np.ascontiguousarray(wzr, dtype=bf)
    wht = np.ascontiguousarray(wht, dtype=bf)
    wlin = np.ascontiguousarray(wlin, dtype=bf)
    ident = np.eye(128, dtype=bf)

    in_maps = []
    for i in range(NCORES):
        sl = slice(i * BL, (i + 1) * BL)
        gf = gam[:, sl, :]  # (T,BL,H)
        # gam packed: [p, t, kt*BL+b]
        gp = np.ascontiguousarray(
            gf.reshape(T, BL, KC, 128).transpose(3, 0, 2, 1).reshape(128, T, KC * BL),
            dtype=bf)

        def packA(A):
            return np.ascontiguousarray(
                A[sl].reshape(BL, JT, 128).transpose(2, 1, 0).reshape(128, JT * BL), dtype=bf)

        in_maps.append({
            "gam": gp,
            "wzr": wzr,
            "wht": wht,
            "wlin": wlin,
            "a0z": packA(Az0),
            "a0r": packA(Ar0),
            "a0h": packA(Ah0),
            "ident": ident,
        })
    return in_maps


def kernel(C, t, mask, Wz, bz, Wr, br, Wh, bh, Wgh, bgh, wgx, bgx, Wlin, blin,
           _trace=False, _trace_kwargs=None):
    C = np.asarray(C, np.float32)
    t = np.asarray(t, np.float32)
    nc = _build_program()
    in_maps = _host_prep(C, t,
                         np.asarray(Wz, np.float32), np.asarray(bz, np.float32),
                         np.asarray(Wr, np.float32), np.asarray(br, np.float32),
                         np.asarray(Wh, np.float32), np.asarray(bh, np.float32),
                         np.asarray(Wgh, np.float32), np.asarray(bgh, np.float32),
                         np.asarray(Wlin, np.float32))

    from concourse.bass_utils import run_bass_kernel_spmd
    res = run_bass_kernel_spmd(nc, in_maps, list(range(NCORES)),
                               trace=_trace, **(_trace_kwargs or {}))
    outs = [res.results[i]["out"] for i in range(NCORES)]
    full = np.concatenate(outs, axis=1).astype(np.float32)  # (T,B,O)
    full += np.asarray(blin, np.float32)[None, None, :]
    kernel._last_results = res
    return full
